# revision 1
# baseline (speedup 1.0000x reference)
"""Trainium2 Bass kernel for AttnProcessor self-attention (B=2,S=2048,C=1024,H=16).

Sharding: 8 cores, core c owns heads (2c, 2c+1) for both batches (tensor
parallel on the head dim for QKV); attention outputs are redistributed with
two 8-core AllToAlls (one per batch, the first hidden under batch-1 compute)
so core c computes the output projection + residual for output rows
(b=c//4, s in [512*(c%4), 512*(c%4+1))). Host picks out1/out2 per core.

Per-core pipeline (all matmuls fp32r):
  qT/kT projections in [c'=128, s] layout, v' in [s, 2x(64+ones)] layout
  (padded to 256 free for fp32r full rate), row-tiled (64x128) QK^T per head
  pair, exp on ScalarE (scale=1/8, no max subtraction -- scores are O(5)),
  PV accumulates V'.T @ probsT giving both the attention output (transposed)
  and the softmax denominators (ones row). Denominator reciprocals are
  computed batched pre-collective and ride the AllToAll; normalization +
  output projection run per received chunk.
"""
import numpy as np

import concourse.bacc as bacc
import concourse.bass as bass
import concourse.tile as tile
import concourse.tile_rust as tile_rust
from concourse import mybir
from concourse.bass_utils import run_bass_kernel_spmd

F32 = mybir.dt.float32
F32R = mybir.dt.float32r

B, S, C, H, D = 2, 2048, 1024, 16, 64
N_CORES = 8
BS = B * S  # 4096
SCALE = 1.0 / np.sqrt(D)

_CACHE = {}


def _build():
    nc = bacc.Bacc(num_devices=N_CORES)
    hsT = nc.declare_dram_parameter("hsT", [C, BS], F32R, isOutput=False)
    wq = nc.declare_dram_parameter("wq", [C, 128], F32R, isOutput=False)
    wk = nc.declare_dram_parameter("wk", [C, 128], F32R, isOutput=False)
    wv = nc.declare_dram_parameter("wv", [C, 256], F32R, isOutput=False)
    wo = nc.declare_dram_parameter("wo", [C, C], F32R, isOutput=False)
    bqk = nc.declare_dram_parameter("bqk", [128, 2], F32, isOutput=False)
    bvb = nc.declare_dram_parameter("bvb", [1, 256], F32, isOutput=False)
    res = nc.declare_dram_parameter("res", [512, C], F32, isOutput=False)
    out1 = nc.declare_dram_parameter("out1", [256, C], F32, isOutput=True)
    out2 = nc.declare_dram_parameter("out2", [256, C], F32, isOutput=True)

    with tile.TileContext(nc) as tc:
        with (
            tc.tile_pool(name="wpool", bufs=1) as wpool,
            tc.tile_pool(name="hpool", bufs=1) as hpool,
            tc.tile_pool(name="qkpool", bufs=2) as qkpool,
            tc.tile_pool(name="ppool", bufs=3) as ppool,
            tc.tile_pool(name="spool", bufs=3) as spool,
            tc.tile_pool(name="opool", bufs=2) as opool,
            tc.tile_pool(name="psum", bufs=1, space="PSUM") as psum,
            tc.tile_pool(name="dram", bufs=1, space="DRAM") as dram,
        ):
            # ---- weight / constant / input loads ----
            # single strided DMA per weight tensor: sbuf [128, 8*N] with
            # chunk cc at cols N*cc  <-  dram [1024, N]
            wo_sb = []

            def load_w(name, src, ncols):
                t = wpool.tile([128, 8 * ncols], F32R, tag=name)
                sap = src[:]
                nc.scalar.dma_start(
                    out=t[:],
                    in_=bass.AP(tensor=sap.tensor, offset=sap.offset,
                                ap=[[ncols, 128], [128 * ncols, 8],
                                    [1, ncols]]))
                return [t[:, ncols * cc:ncols * (cc + 1)] for cc in range(8)]

            wq_sb = load_w("wq", wq, 128)
            hs0 = []
            for cc in range(8):
                t = hpool.tile([128, 2048], F32R, tag=f"hs{cc}", name=f"hs0_{cc}")
                hs0.append(t)
            for g in range(2):
                for cc in range(8):
                    eng = nc.scalar if g == 0 else nc.sync
                    eng.dma_start(
                        out=hs0[cc][:, 1024 * g:1024 * (g + 1)],
                        in_=hsT[128 * cc:128 * (cc + 1),
                                1024 * g:1024 * (g + 1)])
            wk_sb = load_w("wk", wk, 128)
            wv_sb = load_w("wv", wv, 256)
            bqk_sb = wpool.tile([128, 2], F32, tag="bqk")
            nc.scalar.dma_start(out=bqk_sb[:], in_=bqk[:])
            bvb_sb = wpool.tile([128, 256], F32, tag="bvb")
            bvb_ap = bvb[:]
            nc.scalar.dma_start(
                out=bvb_sb[:],
                in_=bass.AP(tensor=bvb_ap.tensor, offset=bvb_ap.offset,
                            ap=[[0, 128], [1, 256]]),
            )

            a2a_in = [dram.tile([8, 130, 256], F32R, name=f"a2ain{b}")
                      for b in range(2)]
            a2a_out = [dram.tile([8, 130, 256], F32R, name=f"a2aout{b}")
                       for b in range(2)]

            qT, kT, vS, sums_pre = {}, {}, {}, {}
            last_drain = [None]

            def emit_hsT_load(b):
                tiles = []
                for cc in range(8):
                    t = hpool.tile([128, 2048], F32R, tag=f"hs{cc}",
                                   name=f"hs{b}_{cc}")
                    nc.scalar.dma_start(
                        out=t[:],
                        in_=hsT[128 * cc:128 * (cc + 1), 2048 * b:2048 * (b + 1)])
                    tiles.append(t)
                return tiles

            def emit_proj_qk(b, hs_sb, t_idx, j):
                """One unit: tensor t_idx (0=q,1=k), one 512-wide s-slice j."""
                if t_idx == 0:
                    if b not in qT:
                        qT[b] = qkpool.tile([128, 2048], F32R, tag="qT",
                                            name=f"qT{b}")
                    dst, w_sb = qT[b], wq_sb
                else:
                    if b not in kT:
                        kT[b] = qkpool.tile([128, 2048], F32R, tag="kT",
                                            name=f"kT{b}")
                    dst, w_sb = kT[b], wk_sb
                ps = psum.tile([128, 512], F32, tag="big", bufs=3,
                               name=f"pqk{b}_{t_idx}_{j}")
                for cc in range(8):
                    nc.tensor.matmul(
                        ps[:], w_sb[cc],
                        hs_sb[cc][:, 512 * j:512 * (j + 1)],
                        start=(cc == 0), stop=(cc == 7))
                nc.vector.tensor_scalar_add(
                    out=dst[:, 512 * j:512 * (j + 1)], in0=ps[:],
                    scalar1=bqk_sb[:, t_idx:t_idx + 1])

            def emit_proj_v(b, hs_sb, i):
                """One unit: one 128-row v' s-tile i."""
                if b not in vS:
                    vS[b] = qkpool.tile([128, 2080], F32R, tag="vS",
                                        name=f"vS{b}")
                dst = vS[b]
                ps = psum.tile([128, 512], F32, tag="big", bufs=3,
                               name=f"pv{b}_{i}")
                sl = ps[:, 0:256]
                for cc in range(8):
                    nc.tensor.matmul(
                        sl, hs_sb[cc][:, 128 * i:128 * (i + 1)], wv_sb[cc],
                        start=(cc == 0), stop=(cc == 7))
                nc.vector.tensor_tensor(
                    out=dst[:, 130 * i:130 * (i + 1)], in0=sl[:, 0:130],
                    in1=bvb_sb[:, 0:130], op=mybir.AluOpType.add)

            def emit_attention_qs(b, qs, fill_work):
                """One q-slice (512 q) for both heads, processed in kc-pairs:
                per step, fills then 2 exps, then 4 QK mms (64-row config),
                then 4 PV mms (128-row config, bank-paired A,A,B,B)."""
                accA = psum.tile([65, 512], F32, tag="accA", bufs=1,
                                 name=f"accA_{b}_{qs}")
                accB = psum.tile([65, 512], F32, tag="accB", bufs=1,
                                 name=f"accB_{b}_{qs}")
                sc_t = {}

                def emit_qk(kc):
                    sc = psum.tile([128, 1024], F32, tag="big", bufs=3,
                                   name=f"sc_{b}_{qs}_{kc}")
                    sc_t[kc] = sc
                    nc.tensor.matmul(
                        sc[:, 0:512],
                        kT[b][0:64, 128 * kc:128 * (kc + 1)],
                        qT[b][0:64, 512 * qs:512 * (qs + 1)],
                        start=True, stop=True)
                    nc.tensor.matmul(
                        sc[:, 512:1024],
                        kT[b][64:128, 128 * kc:128 * (kc + 1)],
                        qT[b][64:128, 512 * qs:512 * (qs + 1)],
                        start=True, stop=True)

                def emit_pv(acc, off, kc, pr):
                    nc.tensor.matmul(
                        acc[:],
                        vS[b][:, 130 * kc + off:130 * kc + off + 65],
                        pr[:, (0 if off == 0 else 512):
                           (512 if off == 0 else 1024)],
                        start=(kc == 0), stop=(kc == 15))

                emit_qk(0)
                emit_qk(1)
                for step in range(8):
                    kc0, kc1 = 2 * step, 2 * step + 1
                    for _ in range(2):
                        if fill_work:
                            fill_work.pop(0)()
                    pr0 = ppool.tile([128, 1024], F32R, tag="pr", bufs=4,
                                     name=f"pr_{b}_{qs}_{kc0}")
                    nc.scalar.activation(pr0[:], sc_t.pop(kc0)[:],
                                         mybir.ActivationFunctionType.Exp,
                                         scale=float(SCALE))
                    pr1 = ppool.tile([128, 1024], F32R, tag="pr", bufs=4,
                                     name=f"pr_{b}_{qs}_{kc1}")
                    nc.scalar.activation(pr1[:], sc_t.pop(kc1)[:],
                                         mybir.ActivationFunctionType.Exp,
                                         scale=float(SCALE))
                    if step < 7:
                        emit_qk(kc0 + 2)
                        emit_qk(kc1 + 2)
                    emit_pv(accA, 0, kc0, pr0)
                    emit_pv(accA, 0, kc1, pr1)
                    emit_pv(accB, 65, kc0, pr0)
                    emit_pv(accB, 65, kc1, pr1)
                # drain: rows [0:64] -> a2a_in, row 64 (sums) -> sums_pre
                if b not in sums_pre:
                    sums_pre[b] = opool.tile([8, 512], F32R, tag="sums",
                                             name=f"sums{b}")
                for h, acc in ((0, accA), (1, accB)):
                    st = spool.tile([65, 512], F32R, tag="st",
                                    name=f"st_{b}_{qs}_{h}")
                    nc.vector.tensor_copy(st[:], acc[:])
                    for half in range(2):
                        d = nc.sync.dma_start(
                            out=a2a_in[b][2 * qs + half,
                                          64 * h:64 * (h + 1), :],
                            in_=st[0:64, 256 * half:256 * (half + 1)])
                        last_drain[0] = d
                    nc.sync.dma_start(
                        out=sums_pre[b][2 * qs + h:2 * qs + h + 1, :],
                        in_=st[64:65, :])

            def emit_recip_ship(b):
                with nc.allow_low_precision("f32r softmax denominators"):
                    nc.vector.reciprocal(sums_pre[b][:],
                                         sums_pre[b][:].bitcast(F32))
                for qs in range(4):
                    for h in range(2):
                        for half in range(2):
                            nc.sync.dma_start(
                                out=a2a_in[b][2 * qs + half,
                                              128 + h:129 + h, :],
                                in_=sums_pre[b][2 * qs + h:2 * qs + h + 1,
                                                256 * half:256 * (half + 1)])

            def emit_collective(b):
                nc.gpsimd.collective_compute(
                    "AllToAll", mybir.AluOpType.bypass,
                    replica_groups=[list(range(8))],
                    ins=[a2a_in[b][:]], outs=[a2a_out[b][:]])

            def emit_output(b, out_t, res_pair, after=None):
                """Normalize received [128,256] chunks, outproj 2 stiles."""
                op_ps = []
                for st_i in range(2):
                    ps = psum.tile([128, 1024], F32, tag="big", bufs=3,
                                   name=f"op{b}_{st_i}")
                    op_ps.append((ps[:, 0:512], ps[:, 512:1024], ps))
                for j in range(8):
                    raw = opool.tile([128, 256], F32, tag="raw",
                                     name=f"raw{b}_{j}")
                    rd = nc.scalar.dma_start(out=raw[:],
                                             in_=a2a_out[b][j, 0:128, :]
                                             .bitcast(F32))
                    if after is not None and j == 0:
                        tile_rust.add_dep_helper(
                            rd.ins, after.ins, False,
                            "hold output norm until attention drained")
                    rbc = opool.tile([128, 256], F32, tag="rbc",
                                     name=f"rbc{b}_{j}")
                    for h in range(2):
                        srow = a2a_out[b][j, 128 + h:129 + h, :].bitcast(F32)
                        nc.scalar.dma_start(
                            out=rbc[64 * h:64 * (h + 1), :],
                            in_=bass.AP(tensor=srow.tensor, offset=srow.offset,
                                        ap=[[0, 64], [1, 256]]))
                    an_t = opool.tile([128, 256], F32R, tag="an",
                                      name=f"an{b}_{j}")
                    an = an_t[:]
                    nc.vector.tensor_tensor(out=an, in0=raw[:], in1=rbc[:],
                                            op=mybir.AluOpType.mult)
                    for st_i in range(2):
                        for co in range(2):
                            nc.tensor.matmul(
                                op_ps[st_i][co],
                                an[:, 128 * st_i:128 * (st_i + 1)],
                                wo_sb[j][:, 512 * co:512 * (co + 1)],
                                start=(j == 0), stop=(j == 7))
                for st_i in range(2):
                    ob = opool.tile([128, 1024], F32, tag="ob",
                                    name=f"ob{b}_{st_i}")
                    nc.vector.tensor_tensor(out=ob[:], in0=op_ps[st_i][2][:],
                                            in1=res_pair[st_i][:],
                                            op=mybir.AluOpType.add)
                    nc.sync.dma_start(
                        out=out_t[128 * st_i:128 * (st_i + 1), 0:512],
                        in_=ob[:, 0:512])
                    nc.scalar.dma_start(
                        out=out_t[128 * st_i:128 * (st_i + 1), 512:1024],
                        in_=ob[:, 512:1024])

            # ---------------- emission ----------------
            # prefix: just enough b0 projection for attention(b0, qs0) kc 0-3
            emit_proj_qk(0, hs0, 0, 0)
            emit_proj_qk(0, hs0, 1, 0)
            for i in range(4):
                emit_proj_v(0, hs0, i)

            hs1 = emit_hsT_load(1)

            def qk_u(b, hs, t, j):
                return lambda: emit_proj_qk(b, hs, t, j)

            def v_u(b, hs, i):
                return lambda: emit_proj_v(b, hs, i)

            # qs0 fill: 2 pops per step-start; each unit lands before its
            # first consumer (vS stile i -> PV at step i//2; kT unit j ->
            # QK(4j) emitted at step 2j-1; deadlines checked offline)
            fill = [qk_u(0, hs0, 1, 1), v_u(0, hs0, 4), v_u(0, hs0, 5),
                    qk_u(0, hs0, 1, 2), v_u(0, hs0, 6), v_u(0, hs0, 7),
                    v_u(0, hs0, 8), v_u(0, hs0, 9), qk_u(0, hs0, 1, 3),
                    v_u(0, hs0, 10), v_u(0, hs0, 11), v_u(0, hs0, 12),
                    v_u(0, hs0, 13), v_u(0, hs0, 14), v_u(0, hs0, 15),
                    qk_u(0, hs0, 0, 1)]
            emit_attention_qs(0, 0, fill)
            fill = [qk_u(0, hs0, 0, 2), qk_u(0, hs0, 0, 3)]
            emit_attention_qs(0, 1, fill)
            fill = []
            for t_idx in range(2):
                for j in range(4):
                    if t_idx == 0 and j >= 2:
                        continue  # deferred into attention(b1) slack
                    fill.append(qk_u(1, hs1, t_idx, j))
            for i in range(16):
                fill.append(v_u(1, hs1, i))
            emit_attention_qs(0, 2, fill)
            emit_attention_qs(0, 3, fill)
            while fill:
                fill.pop(0)()
            emit_recip_ship(0)
            emit_collective(0)

            # load wo / res during attention(b1); reuse freed slots
            for cc in range(8):
                t = hpool.tile([128, 1024], F32R, tag=f"hs{cc}",
                               name=f"wo{cc}")
                nc.sync.dma_start(out=t[:], in_=wo[128 * cc:128 * (cc + 1), :])
                wo_sb.append(t)
            res_sb = []
            for st_i in range(4):
                t = wpool.tile([128, 1024], F32, tag=f"res{st_i}",
                               name=f"res{st_i}")
                nc.sync.dma_start(out=t[:],
                                  in_=res[128 * st_i:128 * (st_i + 1), :])
                res_sb.append(t)

            fill_b1 = [qk_u(1, hs1, 0, 2), qk_u(1, hs1, 0, 3)]
            for qs in range(4):
                emit_attention_qs(1, qs, fill_b1)
            emit_recip_ship(1)
            emit_output(0, out1, res_sb[0:2], after=last_drain[0])
            emit_collective(1)
            emit_output(1, out2, res_sb[2:4])
    nc.finalize()
    return nc


def _prep_inputs(hidden_states, Wq, bq, Wk, bk, Wv, bv, Wo, bo):
    hs = np.asarray(hidden_states, np.float32)
    hsT = np.ascontiguousarray(
        hs.transpose(2, 0, 1).reshape(C, BS)).astype(np.float32)
    Wo_f = np.ascontiguousarray(np.asarray(Wo, np.float32))
    in_maps = []
    for c in range(N_CORES):
        h0 = 2 * c
        cols = slice(64 * h0, 64 * h0 + 128)
        wv_c = np.zeros((C, 256), np.float32)
        bvb_c = np.zeros((1, 256), np.float32)
        for a in range(2):
            hd = slice(64 * (h0 + a), 64 * (h0 + a + 1))
            wv_c[:, 65 * a:65 * a + 64] = np.asarray(Wv, np.float32)[:, hd]
            bvb_c[0, 65 * a:65 * a + 64] = np.asarray(bv, np.float32)[hd]
            bvb_c[0, 65 * a + 64] = 1.0
        bqk_c = np.stack([np.asarray(bq, np.float32)[cols],
                          np.asarray(bk, np.float32)[cols]], axis=1)
        s0 = 256 * c
        bo_f = np.asarray(bo, np.float32)
        res_c = np.concatenate(
            [hs[0, s0:s0 + 256, :] + bo_f, hs[1, s0:s0 + 256, :] + bo_f],
            axis=0).astype(np.float32)
        in_maps.append({
            "hsT": hsT,
            "wq": np.ascontiguousarray(np.asarray(Wq, np.float32)[:, cols]),
            "wk": np.ascontiguousarray(np.asarray(Wk, np.float32)[:, cols]),
            "wv": wv_c,
            "wo": Wo_f,
            "bqk": np.ascontiguousarray(bqk_c),
            "bvb": bvb_c,
            "res": np.ascontiguousarray(res_c),
        })
    return in_maps


def _run(inputs, trace=False, trace_kwargs=None):
    if "nc" not in _CACHE:
        _CACHE["nc"] = _build()
    nc = _CACHE["nc"]
    in_maps = _prep_inputs(**inputs)
    r = run_bass_kernel_spmd(nc, in_maps, core_ids=list(range(N_CORES)),
                             trace=trace, **(trace_kwargs or {}))
    full = np.empty((B, S, C), np.float32)
    for c in range(N_CORES):
        full[0, 256 * c:256 * (c + 1), :] = r.results[c]["out1"]
        full[1, 256 * c:256 * (c + 1), :] = r.results[c]["out2"]
    return full, r


def kernel(**inputs):
    full, _ = _run(inputs, trace=False)
    return full



# revision 5
# speedup vs baseline: 1.4597x; 1.4597x over previous
"""Trainium2 Bass kernel for AttnProcessor self-attention (B=2,S=2048,C=1024,H=16).

Sharding: 8 cores, core c owns heads (2c, 2c+1) for both batches (tensor
parallel on the head dim for QKV). All matmul inputs are bf16 (fp32 psum
accumulation); rel err ~5e-4 vs the 2e-2 gate.

Token->core output mapping is interleaved so each 512-token q-slice (qs)
contains one 64-token block for every destination core: core c owns tokens
512*qs + 64*c .. +64 of every (b, qs). Each (b, qs) ships its attention
output in its own small AllToAll ([8,132,64] bf16 = 135KB) right after that
q-slice's PV completes - 8 pipelined collectives instead of 2 big late
ones. Payload rows 0-127 are the two heads' attnout.T in bf16; rows
128-131 carry the f32 softmax denominators (bitcast); reciprocals are
computed receiver-side with reciprocal_approx_fast. Output projection runs
per qs-pair as PE fill work inside later attention slices.

Per-core pipeline: qT/kT projections in [c'=128, s] layout (256-wide fill
units), v' in [s, 2x(64+ones)] = 130-wide layout, per-kc QK -> exp
(ScalarE, scale=1/8, no max subtraction) -> PV (accumulates attnout.T and
softmax denominators via the ones column). ScalarE runs ONLY exps; DMAs
are issued from sync (loads/recv reads/out writes) and gpsimd
(drains/collectives); DVE does casts/recips/biases/normalization.
"""
import numpy as np

import concourse.bacc as bacc
import concourse.bass as bass
import concourse.tile as tile
from concourse import mybir
from concourse.bass_utils import run_bass_kernel_spmd

F32 = mybir.dt.float32
BF16 = mybir.dt.bfloat16

B, S, C, H, D = 2, 2048, 1024, 16, 64
N_CORES = 8
BS = B * S  # 4096
SCALE = 1.0 / np.sqrt(D)

# a2a chunk geometry (per destination core): 132 rows x 64 tokens bf16
CH_BF = 132 * 64          # bf16 elems per dest chunk (8448)
CH_F32 = CH_BF // 2       # f32 elems per dest chunk (4224)

_CACHE = {}


def _build():
    nc = bacc.Bacc(num_devices=N_CORES)
    hsT = nc.declare_dram_parameter("hsT", [C, BS], BF16, isOutput=False)
    wq = nc.declare_dram_parameter("wq", [C, 128], BF16, isOutput=False)
    wk = nc.declare_dram_parameter("wk", [C, 128], BF16, isOutput=False)
    wv = nc.declare_dram_parameter("wv", [C, 130], BF16, isOutput=False)
    wo = nc.declare_dram_parameter("wo", [C, C], BF16, isOutput=False)
    bqk = nc.declare_dram_parameter("bqk", [128, 2], F32, isOutput=False)
    bvb = nc.declare_dram_parameter("bvb", [1, 130], F32, isOutput=False)
    res = nc.declare_dram_parameter("res", [512, C], F32, isOutput=False)
    out1 = nc.declare_dram_parameter("out1", [256, C], F32, isOutput=True)
    out2 = nc.declare_dram_parameter("out2", [256, C], F32, isOutput=True)

    with tile.TileContext(nc) as tc:
        with (
            tc.tile_pool(name="wpool", bufs=1) as wpool,
            tc.tile_pool(name="hpool", bufs=1) as hpool,
            tc.tile_pool(name="qkpool", bufs=2) as qkpool,
            tc.tile_pool(name="ppool", bufs=4) as ppool,
            tc.tile_pool(name="spool", bufs=4) as spool,
            tc.tile_pool(name="opool", bufs=2) as opool,
            tc.tile_pool(name="psum", bufs=1, space="PSUM") as psum,
            tc.tile_pool(name="dram", bufs=1, space="DRAM") as dram,
        ):
            # ---- weight / constant / input loads (sync queue) ----
            def load_w(name, src, ncols):
                t = wpool.tile([128, 8 * ncols], BF16, tag=name)
                sap = src[:]
                nc.sync.dma_start(
                    out=t[:],
                    in_=bass.AP(tensor=sap.tensor, offset=sap.offset,
                                ap=[[ncols, 128], [128 * ncols, 8],
                                    [1, ncols]]))
                return [t[:, ncols * cc:ncols * (cc + 1)] for cc in range(8)]

            wq_sb = load_w("wq", wq, 128)
            wk_sb = load_w("wk", wk, 128)
            wv_sb = load_w("wv", wv, 130)
            bqk_sb = wpool.tile([128, 2], F32, tag="bqk")
            nc.sync.dma_start(out=bqk_sb[:], in_=bqk[:])
            bvb_sb = wpool.tile([128, 130], F32, tag="bvb")
            bvb_ap = bvb[:]
            nc.sync.dma_start(
                out=bvb_sb[:],
                in_=bass.AP(tensor=bvb_ap.tensor, offset=bvb_ap.offset,
                            ap=[[0, 128], [1, 130]]))

            def emit_hsT_load(b):
                tiles = []
                for cc in range(8):
                    t = hpool.tile([128, 2048], BF16, tag=f"hs{cc}",
                                   name=f"hs{b}_{cc}")
                    nc.sync.dma_start(
                        out=t[:],
                        in_=hsT[128 * cc:128 * (cc + 1),
                                2048 * b:2048 * (b + 1)])
                    tiles.append(t)
                return tiles

            hs0 = emit_hsT_load(0)

            a2a_in = [[dram.tile([8, 132, 64], BF16, name=f"a2ain{b}_{qs}")
                       for qs in range(4)] for b in range(2)]
            a2a_out = [[dram.tile([8, 132, 64], BF16, name=f"a2aout{b}_{qs}")
                        for qs in range(4)] for b in range(2)]

            qT, kT, vS = {}, {}, {}

            def emit_proj_qk(b, hs_sb, t_idx, j2):
                """One unit: tensor t_idx (0=q,1=k), one 256-wide s-slice."""
                if t_idx == 0:
                    if b not in qT:
                        qT[b] = qkpool.tile([128, 2048], BF16, tag="qT",
                                            name=f"qT{b}")
                    dst, w_sb = qT[b], wq_sb
                else:
                    if b not in kT:
                        kT[b] = qkpool.tile([128, 2048], BF16, tag="kT",
                                            name=f"kT{b}")
                    dst, w_sb = kT[b], wk_sb
                ps = psum.tile([128, 512], F32, tag="aux", bufs=2,
                               name=f"pqk{b}_{t_idx}_{j2}")
                sl = ps[:, 0:256]
                for cc in range(8):
                    nc.tensor.matmul(
                        sl, w_sb[cc],
                        hs_sb[cc][:, 256 * j2:256 * (j2 + 1)],
                        start=(cc == 0), stop=(cc == 7))
                nc.vector.tensor_scalar_add(
                    out=dst[:, 256 * j2:256 * (j2 + 1)], in0=sl,
                    scalar1=bqk_sb[:, t_idx:t_idx + 1])

            def emit_proj_v(b, hs_sb, i):
                """One unit: one 128-row v' s-tile i (130 cols, bf16)."""
                if b not in vS:
                    vS[b] = qkpool.tile([128, 2080], BF16, tag="vS",
                                        name=f"vS{b}")
                ps = psum.tile([128, 512], F32, tag="aux", bufs=2,
                               name=f"pv{b}_{i}")
                sl = ps[:, 0:130]
                for cc in range(8):
                    nc.tensor.matmul(
                        sl, hs_sb[cc][:, 128 * i:128 * (i + 1)], wv_sb[cc],
                        start=(cc == 0), stop=(cc == 7))
                nc.vector.tensor_tensor(
                    out=vS[b][:, 130 * i:130 * (i + 1)], in0=sl,
                    in1=bvb_sb[:, 0:130], op=mybir.AluOpType.add)

            def emit_attention_qs(b, qs, fill_work):
                """One q-slice (512 q) for both heads; 16 kc steps.
                Per step: 2 fills, exp(kc), QK(kc+1), PV(kc)x2."""
                accA = psum.tile([65, 512], F32, tag="accA", bufs=1,
                                 name=f"accA_{b}_{qs}")
                accB = psum.tile([65, 512], F32, tag="accB", bufs=1,
                                 name=f"accB_{b}_{qs}")
                sc_t = {}

                def emit_qk(kc):
                    sc = psum.tile([128, 1024], F32, tag="sc", bufs=2,
                                   name=f"sc_{b}_{qs}_{kc}")
                    sc_t[kc] = sc
                    nc.tensor.matmul(
                        sc[:, 0:512],
                        kT[b][0:64, 128 * kc:128 * (kc + 1)],
                        qT[b][0:64, 512 * qs:512 * (qs + 1)],
                        start=True, stop=True)
                    nc.tensor.matmul(
                        sc[:, 512:1024],
                        kT[b][64:128, 128 * kc:128 * (kc + 1)],
                        qT[b][64:128, 512 * qs:512 * (qs + 1)],
                        start=True, stop=True)

                emit_qk(0)
                for kc in range(16):
                    for _ in range(2):
                        if fill_work:
                            fill_work.pop(0)()
                    pr = ppool.tile([128, 1024], BF16, tag="pr", bufs=4,
                                    name=f"pr_{b}_{qs}_{kc}")
                    nc.scalar.activation(pr[:], sc_t.pop(kc)[:],
                                         mybir.ActivationFunctionType.Exp,
                                         scale=float(SCALE))
                    if kc < 15:
                        emit_qk(kc + 1)
                    nc.tensor.matmul(
                        accA[:], vS[b][:, 130 * kc:130 * kc + 65],
                        pr[:, 0:512], start=(kc == 0), stop=(kc == 15))
                    nc.tensor.matmul(
                        accB[:], vS[b][:, 130 * kc + 65:130 * kc + 130],
                        pr[:, 512:1024], start=(kc == 0), stop=(kc == 15))

                # drain: cast attnout to bf16, ship per-dest + f32 sums rows
                a2a_t = a2a_in[b][qs][:]
                a2a_f = a2a_t.bitcast(F32)
                for h, acc in ((0, accA), (1, accB)):
                    st = spool.tile([64, 512], BF16, tag="st",
                                    name=f"st_{b}_{qs}_{h}")
                    nc.vector.tensor_copy(st[:], acc[0:64, :])
                    stp = st[:]
                    # payload: st[0:64, 64*d+t] -> a2a[d, 64h+r, t]
                    nc.gpsimd.dma_start(
                        out=bass.AP(tensor=a2a_t.tensor, offset=a2a_t.offset
                                    + 64 * h * 64,
                                    ap=[[64, 64], [CH_BF, 8], [1, 64]]),
                        in_=bass.AP(tensor=stp.tensor, offset=stp.offset,
                                    ap=[list(stp.ap[0]), [64, 8], [1, 64]]))
                    # f32 sums: acc[64, 64*d+t] -> f32 rows 128+2h..129+2h
                    sm_sb = spool.tile([1, 512], F32, tag="sm",
                                       name=f"sm_{b}_{qs}_{h}")
                    nc.vector.tensor_copy(sm_sb[:], acc[64:65, :])
                    sm = sm_sb[:]
                    nc.gpsimd.dma_start(
                        out=bass.AP(tensor=a2a_f.tensor, offset=a2a_f.offset
                                    + (128 + 2 * h) * 32,
                                    ap=[[CH_F32, 8], [1, 64]]),
                        in_=bass.AP(tensor=sm.tensor, offset=sm.offset,
                                    ap=[list(sm.ap[0]), [64, 8], [1, 64]]))
                nc.gpsimd.collective_compute(
                    "AllToAll", mybir.AluOpType.bypass,
                    replica_groups=[list(range(8))],
                    ins=[a2a_in[b][qs][:]], outs=[a2a_out[b][qs][:]])

            # ---- output side ----
            an_all = {}

            def emit_recv(b, p, half):
                """After A2A (b, qs=2p+half): read+normalize into an_all."""
                qs = 2 * p + half
                a2a_t = a2a_out[b][qs][:]
                a2a_f = a2a_t.bitcast(F32)
                if (b, p) not in an_all:
                    an_all[(b, p)] = opool.tile([128, 1024], BF16, tag="an",
                                                name=f"an{b}_{p}")
                raw = opool.tile([128, 512], BF16, tag="raw",
                                 name=f"raw{b}_{qs}")
                rawap = raw[:]
                nc.sync.dma_start(
                    out=bass.AP(tensor=rawap.tensor, offset=rawap.offset,
                                ap=[list(rawap.ap[0]), [64, 8], [1, 64]]),
                    in_=bass.AP(tensor=a2a_t.tensor, offset=a2a_t.offset,
                                ap=[[64, 128], [CH_BF, 8], [1, 64]]))
                sbc = opool.tile([128, 512], F32, tag="sbc",
                                 name=f"sbc{b}_{qs}")
                for h in range(2):
                    sbch = sbc[64 * h:64 * (h + 1), :]
                    nc.sync.dma_start(
                        out=bass.AP(tensor=sbch.tensor, offset=sbch.offset,
                                    ap=[list(sbch.ap[0]), [64, 8], [1, 64]]),
                        in_=bass.AP(tensor=a2a_f.tensor, offset=a2a_f.offset
                                    + (128 + 2 * h) * 32,
                                    ap=[[0, 64], [CH_F32, 8], [1, 64]]))
                rbc = opool.tile([128, 512], F32, tag="rbc",
                                 name=f"rbc{b}_{qs}")
                nc.vector.reciprocal_approx_fast(rbc[:], sbc[:])
                # an[:, 128j + 64*half + t] = raw[:, 64j+t] * rbc[:, 64j+t]
                anap = an_all[(b, p)][:]
                rbcap = rbc[:]
                nc.vector.tensor_tensor(
                    out=bass.AP(tensor=anap.tensor,
                                offset=anap.offset + 64 * half,
                                ap=[list(anap.ap[0]), [128, 8], [1, 64]]),
                    in0=bass.AP(tensor=rawap.tensor, offset=rawap.offset,
                                ap=[list(rawap.ap[0]), [64, 8], [1, 64]]),
                    in1=bass.AP(tensor=rbcap.tensor, offset=rbcap.offset,
                                ap=[list(rbcap.ap[0]), [64, 8], [1, 64]]),
                    op=mybir.AluOpType.mult)

            wo_sb = []
            res_sb = []

            def emit_out_u(b, p, co):
                """Outproj half: 512 out cols for 128 tokens of pair p."""
                an = an_all[(b, p)]
                ps = psum.tile([128, 512], F32, tag="aux", bufs=2,
                               name=f"op{b}_{p}_{co}")
                for j in range(8):
                    nc.tensor.matmul(
                        ps[:], an[:, 128 * j:128 * (j + 1)],
                        wo_sb[j][:, 512 * co:512 * (co + 1)],
                        start=(j == 0), stop=(j == 7))
                ob = opool.tile([128, 512], F32, tag="ob",
                                name=f"ob{b}_{p}_{co}")
                nc.vector.tensor_tensor(
                    out=ob[:], in0=ps[:],
                    in1=res_sb[2 * b + p][:, 512 * co:512 * (co + 1)],
                    op=mybir.AluOpType.add)
                out_t = out1 if b == 0 else out2
                nc.sync.dma_start(
                    out=out_t[128 * p:128 * (p + 1),
                              512 * co:512 * (co + 1)],
                    in_=ob[:])

            # ---------------- emission ----------------
            # prefix: enough of b0 projections for attention(b0, qs0) start
            emit_proj_qk(0, hs0, 0, 0)
            emit_proj_qk(0, hs0, 0, 1)
            emit_proj_qk(0, hs0, 1, 0)
            emit_proj_v(0, hs0, 0)
            emit_proj_v(0, hs0, 1)

            def qk_u(b, hs, t, j2):
                return lambda: emit_proj_qk(b, hs, t, j2)

            def v_u(b, hs, i):
                return lambda: emit_proj_v(b, hs, i)

            def nop():
                pass

            # qs0 fills: kT j2=1..7 early (deadline step 2*j2-2), v2..v15
            # (deadline step i-1), then qT j2=2..7 (needed by qs1+).
            fill = [qk_u(0, hs0, 1, 1), v_u(0, hs0, 2),
                    qk_u(0, hs0, 1, 2), v_u(0, hs0, 3),
                    qk_u(0, hs0, 1, 3), v_u(0, hs0, 4),
                    qk_u(0, hs0, 1, 4), v_u(0, hs0, 5),
                    qk_u(0, hs0, 1, 5), v_u(0, hs0, 6),
                    qk_u(0, hs0, 1, 6), v_u(0, hs0, 7),
                    qk_u(0, hs0, 1, 7), v_u(0, hs0, 8),
                    v_u(0, hs0, 9), v_u(0, hs0, 10),
                    v_u(0, hs0, 11), v_u(0, hs0, 12),
                    v_u(0, hs0, 13), v_u(0, hs0, 14),
                    v_u(0, hs0, 15), qk_u(0, hs0, 0, 2),
                    qk_u(0, hs0, 0, 3), qk_u(0, hs0, 0, 4),
                    qk_u(0, hs0, 0, 5), qk_u(0, hs0, 0, 6),
                    qk_u(0, hs0, 0, 7)]
            emit_attention_qs(0, 0, fill)
            assert not fill

            # b1 input load (sync queue; lands during b0 qs1/qs2)
            hs1 = emit_hsT_load(1)
            # wo / res load (sync queue)
            for cc in range(8):
                t = hpool.tile([128, 1024], BF16, tag=f"wo{cc}",
                               name=f"wo{cc}")
                nc.sync.dma_start(out=t[:],
                                  in_=wo[128 * cc:128 * (cc + 1), :])
                wo_sb.append(t)
            for st_i in range(4):
                t = wpool.tile([128, 1024], F32, tag=f"res{st_i}",
                               name=f"res{st_i}")
                nc.sync.dma_start(out=t[:],
                                  in_=res[128 * st_i:128 * (st_i + 1), :])
                res_sb.append(t)

            emit_attention_qs(0, 1, [])

            # b1 projections fill b0 qs2/qs3
            fill = []
            for j2 in range(8):
                fill.append(qk_u(1, hs1, 1, j2))
                fill.append(v_u(1, hs1, 2 * (j2 % 4) + (0 if j2 < 4 else 1)))
            emit_attention_qs(0, 2, fill)
            fill = []
            for j2 in range(8):
                fill.append(qk_u(1, hs1, 0, j2))
                fill.append(v_u(1, hs1,
                                8 + 2 * (j2 % 4) + (0 if j2 < 4 else 1)))
            emit_attention_qs(0, 3, fill)

            # outputs for b0 pair0 ready ~during b1 qs0; pair1 ~qs1;
            # b1 pair0 ~qs3; b1 pair1 = tail. Pad fills so out units
            # land mid-slice (avoid blocking the PE queue on the A2A).
            emit_recv(0, 0, 0)
            emit_recv(0, 0, 1)
            fill = [nop] * 8 + [lambda: emit_out_u(0, 0, 0),
                                lambda: emit_out_u(0, 0, 1)]
            emit_attention_qs(1, 0, fill)
            emit_recv(0, 1, 0)
            emit_recv(0, 1, 1)
            fill = [nop] * 8 + [lambda: emit_out_u(0, 1, 0),
                                lambda: emit_out_u(0, 1, 1)]
            emit_attention_qs(1, 1, fill)
            emit_recv(1, 0, 0)
            emit_attention_qs(1, 2, [])
            emit_recv(1, 0, 1)
            fill = [nop] * 8 + [lambda: emit_out_u(1, 0, 0),
                                lambda: emit_out_u(1, 0, 1)]
            emit_attention_qs(1, 3, fill)
            emit_recv(1, 1, 0)
            emit_recv(1, 1, 1)
            emit_out_u(1, 1, 0)
            emit_out_u(1, 1, 1)
    nc.finalize()
    return nc


def _prep_inputs(hidden_states, Wq, bq, Wk, bk, Wv, bv, Wo, bo):
    import ml_dtypes
    bf16 = ml_dtypes.bfloat16
    hs = np.asarray(hidden_states, np.float32)
    hsT = np.ascontiguousarray(
        hs.transpose(2, 0, 1).reshape(C, BS)).astype(bf16)
    Wo_h = np.ascontiguousarray(np.asarray(Wo, np.float32)).astype(bf16)
    bo_f = np.asarray(bo, np.float32)
    in_maps = []
    for c in range(N_CORES):
        h0 = 2 * c
        cols = slice(64 * h0, 64 * h0 + 128)
        wv_c = np.zeros((C, 130), np.float32)
        bvb_c = np.zeros((1, 130), np.float32)
        for a in range(2):
            hd = slice(64 * (h0 + a), 64 * (h0 + a + 1))
            wv_c[:, 65 * a:65 * a + 64] = np.asarray(Wv, np.float32)[:, hd]
            bvb_c[0, 65 * a:65 * a + 64] = np.asarray(bv, np.float32)[hd]
            bvb_c[0, 65 * a + 64] = 1.0
        bqk_c = np.stack([np.asarray(bq, np.float32)[cols],
                          np.asarray(bk, np.float32)[cols]], axis=1)
        # res rows: 128*(2b+p) + 64*half + j  <->  hs[b, 512*(2p+half)+64c+j]
        res_c = np.empty((512, C), np.float32)
        for b in range(2):
            for qs in range(4):
                rows = slice(64 * (4 * b + qs), 64 * (4 * b + qs) + 64)
                toks = slice(512 * qs + 64 * c, 512 * qs + 64 * c + 64)
                res_c[rows] = hs[b, toks, :] + bo_f
        in_maps.append({
            "hsT": hsT,
            "wq": np.ascontiguousarray(
                np.asarray(Wq, np.float32)[:, cols]).astype(bf16),
            "wk": np.ascontiguousarray(
                np.asarray(Wk, np.float32)[:, cols]).astype(bf16),
            "wv": wv_c.astype(bf16),
            "wo": Wo_h,
            "bqk": np.ascontiguousarray(bqk_c),
            "bvb": bvb_c,
            "res": np.ascontiguousarray(res_c),
        })
    return in_maps


def _run(inputs, trace=False, trace_kwargs=None):
    if "nc" not in _CACHE:
        _CACHE["nc"] = _build()
    nc = _CACHE["nc"]
    in_maps = _prep_inputs(**inputs)
    r = run_bass_kernel_spmd(nc, in_maps, core_ids=list(range(N_CORES)),
                             trace=trace, **(trace_kwargs or {}))
    full = np.empty((B, S, C), np.float32)
    for c in range(N_CORES):
        for b in range(2):
            o = r.results[c]["out1" if b == 0 else "out2"]
            for qs in range(4):
                full[b, 512 * qs + 64 * c:512 * qs + 64 * c + 64, :] = \
                    o[64 * qs:64 * qs + 64]
    return full, r


def kernel(**inputs):
    full, _ = _run(inputs, trace=False)
    return full
